# revision 31
# baseline (speedup 1.0000x reference)
"""Causal attention (B=2, T=2048, E=1024, H=16, D=64) on 8 TRN2 NeuronCores.

Sharding: core c handles batch b = c//4 and local head group hg = c%4
(4 heads, 256 head-dims).  Data parallel over batch, tensor parallel over
heads; the output projection is row-parallel, so each core returns a
partial [T, E] output and the host sums the 4 partials per batch (bias
is pre-divided by 4 and added on-device).

v3 design notes (from perfetto traces of v1/v2):
  * ~7us framework preamble and ~8us epilogue are fixed; each dma_start
    costs 0.6-2.5us of issue time on its engine queue, and concurrent
    DMA queues share SDMA bandwidth round-robin (arrival order is only
    loosely tied to issue order).  So: few large DMAs, consumption-
    ordered per ring, with first-needed pieces split small: sync ring
    carries xt (cb-major quarters/halves) then the 16 output stores;
    scalar ring carries wq (halves), wk, wv, wp.
  * mask/ones are generated on device (gpsimd affine_select/memset) and
    bv/bp are broadcast SBUF->SBUF from tiny loads -- saves ~1.2MiB of
    early HBM traffic that was starving the head.
  * 16 warm-up matmuls on a memset tile run during the initial DMA wait
    so the PE HAM clock gate (1.2->2.4GHz after ~3.4us busy) is warm
    when real work arrives.
  * q chains for both pairs run first (chasing the xt cb0 DMA), then k
    chains, then v(t0..3), then attention(ib) per column block with the
    next block's qk/v emitted as ~430ns filler granules popped between
    score(jb+1) and PV(jb) -- the PE FIFO never head-of-line blocks on
    the exp latency.  Projection granules are flexible fill, reserved
    for the late (big) i-blocks and the normalize windows.

Per-core math (all-bf16 matmuls, fp32 PSUM):
  q_t/k_t [hd, t] computed transposed (lhsT=W.T chunk, rhs=xt chunk,
  contraction over e); v natural [t, hd] with 64 ones-columns prepended
  (softmax denominator falls out of the PV matmul on partitions 0:63);
  block-causal scores st[j, i] = q_j . k_i with 2-head row-group
  packing (two K=64 matmuls in distinct PE row halves), exp on ScalarE
  (scale=1/8, no max subtraction -- scores ~N(0,1)), causal mask
  multiply only on block-diagonal tiles, PV accumulation over j in
  PSUM, approx-reciprocal + multiply normalization, then row-parallel
  projection + bp/4.
"""

from collections import deque

import ml_dtypes
import numpy as np

import concourse.bass as bass
import concourse.tile as tile
from concourse import bacc, mybir
from concourse.bass_utils import run_bass_kernel_spmd

B, T, E = 2, 2048, 1024
H, D = 16, 64
NCORES = 8
GROUPS = 4              # cores per batch (tensor parallel over heads)
HL = H // GROUPS        # 4 local heads per core
HDL = HL * D            # 256 local head dims
P = 128
TQ = 512                # i-block (free dim of score tiles)
JB = 128                # j-block (partition dim of score tiles)
N_TB = T // TQ          # 4
N_EC = E // P           # 8
N_TC = T // P           # 16

F32 = mybir.dt.float32
BF16 = mybir.dt.bfloat16
AF = mybir.ActivationFunctionType
ALU = mybir.AluOpType


def _build_nc():
    nc = bacc.Bacc("TRN2", target_bir_lowering=False, debug=False)
    # xt4[cb, hf, p, c, t] = x.T[(hf*4+c)*128 + p, cb*512 + t] -- host
    # pre-shuffled so each (cb, hf) piece is 4KB-contiguous per partition
    # (1KB-line descriptors measured ~2x slower than 4KB lines).
    xt4 = nc.dram_tensor("xt4", [N_TB, 2, P, 4, TQ], BF16,
                         kind="ExternalInput").ap()
    wqt = nc.dram_tensor("wqt", [P, N_EC, HDL], BF16, kind="ExternalInput").ap()
    wkt = nc.dram_tensor("wkt", [P, N_EC, HDL], BF16, kind="ExternalInput").ap()
    wvt = nc.dram_tensor("wvt", [P, N_EC, HDL], BF16, kind="ExternalInput").ap()
    wpt = nc.dram_tensor("wpt", [P, 2, E], BF16, kind="ExternalInput").ap()
    bqv = nc.dram_tensor("bqv", [HDL], F32, kind="ExternalInput").ap()
    bkv = nc.dram_tensor("bkv", [HDL], F32, kind="ExternalInput").ap()
    bvv = nc.dram_tensor("bvv", [HDL], F32, kind="ExternalInput").ap()
    bp4 = nc.dram_tensor("bp4", [E], F32, kind="ExternalInput").ap()
    out = nc.dram_tensor("out", [T, E], BF16, kind="ExternalOutput").ap()

    with tile.TileContext(nc) as tc:
        with (
            tc.tile_pool(name="big", bufs=1) as big,
            tc.tile_pool(name="work", bufs=4) as work,
            tc.tile_pool(name="outp", bufs=3) as outp,
        ):
            # ------------- DMAs: consumption-ordered per ring --------------
            # sync ring: xt pieces, cb-major (cb0 in quarters for fast start)
            xts = [[big.tile([P, 4, TQ], BF16, tag=f"xt{cb}_{hf}",
                             name=f"xt{cb}_{hf}") for hf in range(2)]
                   for cb in range(N_TB)]
            wq_all = big.tile([P, N_EC, HDL], BF16, tag="wq", name="wq")
            wk_all = big.tile([P, N_EC, HDL], BF16, tag="wk", name="wk")
            wv_all = big.tile([P, N_EC, HDL], BF16, tag="wv", name="wv")
            wp_all = big.tile([P, 2, E], BF16, tag="wp", name="wp")
            bv_sb = big.tile([P, HDL], F32, tag="bv", name="bv")
            bp_sb = big.tile([P, E], F32, tag="bp", name="bp")
            # zigzag the global consumption order across the two HWDGE
            # rings: SDMA round-robins between active queues, so arrival
            # order tracks need order only if both ring heads are the
            # next-needed pieces.
            bv_bc = bass.AP(tensor=bvv.tensor, offset=bvv.offset,
                            ap=[[0, P]] + list(bvv.ap))
            bp_bc = bass.AP(tensor=bp4.tensor, offset=bp4.offset,
                            ap=[[0, P]] + list(bp4.ap))
            pieces = [
                (wq_all[:, 0:4, :], wqt[:, 0:4, :]),
                (wq_all[:, 4:8, :], wqt[:, 4:8, :]),
                (xts[0][0][:, 0:2, :], xt4[0, 0][:, 0:2, :]),
                (xts[0][0][:, 2:4, :], xt4[0, 0][:, 2:4, :]),
                (xts[0][1], xt4[0, 1]),
                (wk_all[:, 0:4, :], wkt[:, 0:4, :]),
                (wk_all[:, 4:8, :], wkt[:, 4:8, :]),
                (wv_all[:, 0:4, :], wvt[:, 0:4, :]),
                (wv_all[:, 4:8, :], wvt[:, 4:8, :]),
                (bv_sb, bv_bc),
                (xts[1][0], xt4[1, 0]),
                (xts[1][1], xt4[1, 1]),
                (wp_all, wpt),
                (xts[2][0], xt4[2, 0]),
                (bp_sb, bp_bc),
                (xts[2][1], xt4[2, 1]),
                (xts[3][0], xt4[3, 0]),
                (xts[3][1], xt4[3, 1]),
            ]
            for i, (dst, src) in enumerate(pieces):
                eng = nc.sync if i % 2 == 0 else nc.scalar
                eng.dma_start(dst, src)
            # gpsimd: on-chip mask/ones generation + tiny bias loads
            ones_sb = big.tile([P, HL, D], BF16, tag="ones", name="ones")
            nc.gpsimd.memset(ones_sb, 1.0)
            ones_row = big.tile([1, P], BF16, tag="ones_r", name="ones_r")
            nc.gpsimd.memset(ones_row, 1.0)
            bp_bf = big.tile([1, E], BF16, tag="bp_bf", name="bp_bf")
            bq_sb = big.tile([P, 2], F32, tag="bq", name="bq")
            nc.gpsimd.dma_start(bq_sb, bqv.rearrange("(c p) -> p c", p=P))
            bk_sb = big.tile([P, 2], F32, tag="bk", name="bk")
            nc.gpsimd.dma_start(bk_sb, bkv.rearrange("(c p) -> p c", p=P))
            mask_sb = big.tile([P, GROUPS, TQ], BF16, tag="mask", name="mask")
            nc.gpsimd.memset(mask_sb, 1.0)
            for idx in range(GROUPS):
                # keep 1.0 where (i - p - idx*128) >= 0, else 0.0
                nc.gpsimd.affine_select(
                    out=mask_sb[:, idx, :], in_=mask_sb[:, idx, :],
                    compare_op=ALU.is_ge, fill=0.0,
                    base=-idx * JB, channel_multiplier=-1,
                    pattern=[[1, TQ]])
            nc.gpsimd.tensor_copy(bp_bf, bp_sb[0:1, :])

            q_sb = [big.tile([P, T], BF16, tag=f"q{hc}", name=f"q{hc}")
                    for hc in range(2)]
            k_sb = [big.tile([P, T], BF16, tag=f"k{hc}", name=f"k{hc}")
                    for hc in range(2)]
            at_sb = [big.tile([P, T], BF16, tag=f"at{hc}", name=f"at{hc}")
                     for hc in range(2)]
            v_sb = [big.tile([P, HL, 2 * D], BF16, tag=f"v{t}", name=f"v{t}")
                    for t in range(N_TC)]
            wu_src = big.tile([P, 2 * P], BF16, tag="wusrc", name="wusrc")

            def xt_view(tb, ec):
                return xts[tb][ec // 4][:, ec % 4, :]

            def xt_col(t_, ec):
                cb, c = divmod(t_, 4)
                return xts[cb][ec // 4][:, ec % 4, c * P:(c + 1) * P]

            # ------------- head: warmup + qk(tb0) + v(0..3) ----------------
            # head: only pair-0 q/k and v(t0,t1); pair-1 tb0 chains and
            # v(t2,t3) go through the filler stream inside att(0, 0) so the
            # first score fires as early as the DMA allows.
            with tc.tile_pool(name="ph2", bufs=1, space="PSUM") as ph2:
                nc.vector.memset(wu_src, 0.0)
                wu_ps = ph2.tile([P, 2 * P], F32, tag="wu", name="wu")
                for i in range(22):
                    nc.tensor.matmul(wu_ps, lhsT=wu_src[:, 0:P], rhs=wu_src,
                                     start=(i == 0), stop=(i == 21))
                qp0 = ph2.tile([P, TQ], F32, tag="qp0", name="qp0")
                for ec in range(N_EC):
                    nc.tensor.matmul(
                        qp0, lhsT=wq_all[:, ec, 0:P], rhs=xt_view(0, ec),
                        start=(ec == 0), stop=(ec == N_EC - 1))
                kp0 = ph2.tile([P, TQ], F32, tag="kp0", name="kp0")
                for ec in range(N_EC):
                    nc.tensor.matmul(
                        kp0, lhsT=wk_all[:, ec, 0:P], rhs=xt_view(0, ec),
                        start=(ec == 0), stop=(ec == N_EC - 1))
                nc.vector.tensor_scalar_add(
                    q_sb[0][:, 0:TQ], qp0, bq_sb[:, 0:1])
                nc.vector.tensor_scalar_add(
                    k_sb[0][:, 0:TQ], kp0, bk_sb[:, 0:1])
                for t_ in range(2):
                    vp = ph2.tile([P, HDL], F32, tag="vps", name="vps",
                                  bufs=2)
                    for ec in range(N_EC):
                        nc.tensor.matmul(vp, lhsT=xt_col(t_, ec),
                                         rhs=wv_all[:, ec, :],
                                         start=(ec == 0),
                                         stop=(ec == N_EC - 1))
                    nc.vector.tensor_copy(v_sb[t_][:, :, 0:D], ones_sb)
                    nc.vector.tensor_add(
                        v_sb[t_][:, :, D:2 * D],
                        vp.rearrange("p (h d) -> p h d", h=HL),
                        bv_sb.rearrange("p (h d) -> p h d", h=HL))

            # ------------- streamed attention with filler granules ---------
            with (
                tc.tile_pool(name="stps", bufs=2, space="PSUM") as stps,
                tc.tile_pool(name="accps", bufs=1, space="PSUM") as accps,
                tc.tile_pool(name="mmps", bufs=2, space="PSUM") as mmps,
            ):
                mand = deque()      # qk/v units: must finish within the ib
                flex = deque()      # proj units: any time after deps

                def pop(n=1, allow_flex=True, flex_reserve=0):
                    for _ in range(n):
                        if mand:
                            mand.popleft()()
                        elif allow_flex and len(flex) > flex_reserve:
                            flex.popleft()()

                def flush_mand():
                    while mand:
                        mand.popleft()()

                def unit_qk(hp, wi, tb):
                    w_all = (wq_all, wk_all)[wi]
                    bias_t = (bq_sb, bk_sb)[wi]
                    dst = (q_sb, k_sb)[wi]
                    st = {}

                    def gran(i):
                        def go():
                            if i == 0:
                                st['ps'] = mmps.tile([P, TQ], F32, tag="mm",
                                                     name="mm")
                            for ec in (2 * i, 2 * i + 1):
                                nc.tensor.matmul(
                                    st['ps'],
                                    lhsT=w_all[:, ec, hp * P:(hp + 1) * P],
                                    rhs=xt_view(tb, ec),
                                    start=(ec == 0), stop=(ec == N_EC - 1))
                            if i == 3:
                                nc.vector.tensor_scalar_add(
                                    dst[hp][:, tb * TQ:(tb + 1) * TQ],
                                    st['ps'], bias_t[:, hp:hp + 1])
                        return go
                    return [gran(i) for i in range(4)]

                def unit_v(t_):
                    st = {}

                    def gran(i):
                        def go():
                            if i == 0:
                                st['ps'] = mmps.tile([P, HDL], F32, tag="mm",
                                                     name="mm")
                            for ec in range(4 * i, 4 * i + 4):
                                nc.tensor.matmul(
                                    st['ps'], lhsT=xt_col(t_, ec),
                                    rhs=wv_all[:, ec, :],
                                    start=(ec == 0), stop=(ec == N_EC - 1))
                            if i == 1:
                                nc.vector.tensor_copy(
                                    v_sb[t_][:, :, 0:D], ones_sb)
                                nc.vector.tensor_add(
                                    v_sb[t_][:, :, D:2 * D],
                                    st['ps'].rearrange("p (h d) -> p h d",
                                                       h=HL),
                                    bv_sb.rearrange("p (h d) -> p h d", h=HL))
                        return go
                    return [gran(0), gran(1)]

                def unit_proj(t_, tail=False):
                    st = {}
                    bias_mm = tail and t_ == 15

                    def gran(eb):
                        def go():
                            if eb == 0:
                                st['ot'] = outp.tile([P, E], BF16, tag="ot",
                                                     name="ot")
                            ps = mmps.tile([P, TQ], F32, tag="mm", name="mm")
                            for hc in range(2):
                                nc.tensor.matmul(
                                    ps, lhsT=at_sb[hc][:, t_ * P:(t_ + 1) * P],
                                    rhs=wp_all[:, hc, eb * TQ:(eb + 1) * TQ],
                                    start=(hc == 0), stop=(hc == 1 and
                                                           not bias_mm))
                            osl = st['ot'][:, eb * TQ:(eb + 1) * TQ]
                            if bias_mm:
                                # final chunk, after the last exp: fold the
                                # bias in via a K=1 ones-row matmul and copy
                                # out on the now-idle ScalarE instead of
                                # joining the VectorE tail braid.
                                nc.tensor.matmul(
                                    ps, lhsT=ones_row,
                                    rhs=bp_bf[:, eb * TQ:(eb + 1) * TQ],
                                    start=False, stop=True)
                                nc.scalar.copy(osl, ps)
                                nc.sync.dma_start(
                                    out[t_ * P:(t_ + 1) * P,
                                        eb * TQ:(eb + 1) * TQ], osl)
                                return
                            nc.vector.tensor_add(
                                osl, ps, bp_sb[:, eb * TQ:(eb + 1) * TQ])
                            if t_ == 14:
                                nc.sync.dma_start(
                                    out[t_ * P:(t_ + 1) * P,
                                        eb * TQ:(eb + 1) * TQ], osl)
                            elif eb == 1:
                                nc.sync.dma_start(
                                    out[t_ * P:(t_ + 1) * P, :], st['ot'])
                        return go
                    return [gran(0), gran(1)]

                def attention(hp, ib, popn, allow_flex=True, flex_reserve=0,
                              chunk_hook=None, i0=0, iw=TQ):
                    # processes queries i in [ib*TQ + i0, ib*TQ + i0 + iw)
                    gi0 = ib * TQ + i0
                    njb = (gi0 + iw) // JB
                    nch = iw // JB
                    accs = [accps.tile([2 * D, TQ], F32, tag=f"acc{h}",
                                       name=f"acc{h}") for h in range(2)]

                    def score(jb):
                        idx = (jb * JB - gi0) // JB
                        dd = idx * JB if idx >= 0 else 0
                        stt = stps.tile([P, 2, TQ], F32, tag="st", name="st")
                        for h in range(2):
                            pr = slice(h * D, (h + 1) * D)
                            nc.tensor.matmul(
                                stt[:, h, dd:iw],
                                lhsT=q_sb[hp][pr, jb * JB:(jb + 1) * JB],
                                rhs=k_sb[hp][pr, gi0 + dd:gi0 + iw],
                                start=True, stop=True)
                        pt = work.tile([P, 2, TQ], BF16, tag="pt", name="pt")
                        nc.scalar.activation(pt[:, :, dd:iw], stt[:, :, dd:iw],
                                             AF.Exp, scale=0.125)
                        if idx >= 0:
                            for h in range(2):
                                nc.vector.tensor_mul(
                                    pt[:, h, dd:iw], pt[:, h, dd:iw],
                                    mask_sb[:, idx, dd:iw])
                        return pt, dd

                    def norm_chunk(c0, c1):
                        # normalize at_sb columns [c0*JB, c1*JB) of this
                        # i-range; those acc columns get no further PV
                        # contributions once the diagonal block has passed.
                        w_ = (c1 - c0) * JB
                        for h in range(2):
                            rec64 = work.tile([D, TQ], F32, tag="rec64",
                                              name="rec64", bufs=2)
                            nc.vector.reciprocal_approx_fast(
                                rec64[:, 0:w_],
                                accs[h][0:D, c0 * JB:c1 * JB])
                            nc.vector.tensor_mul(
                                at_sb[hp][h * D:(h + 1) * D,
                                          gi0 + c0 * JB:gi0 + c1 * JB],
                                accs[h][D:2 * D, c0 * JB:c1 * JB],
                                rec64[:, 0:w_])

                    cur = score(0)
                    for jb in range(njb):
                        nxt = score(jb + 1) if jb + 1 < njb else None
                        pop(popn, allow_flex=allow_flex,
                            flex_reserve=flex_reserve)
                        pt, dd = cur
                        for h in range(2):
                            nc.tensor.matmul(
                                accs[h][:, dd:iw],
                                lhsT=v_sb[jb][:, 2 * hp + h, :],
                                rhs=pt[:, h, dd:iw],
                                start=(jb == 0), stop=(jb == njb - 1))
                        idx = (jb * JB - gi0) // JB
                        if chunk_hook is not None and 0 <= idx < nch - 1:
                            # acc cols [0, (idx+1)*JB) final after this block
                            norm_chunk(idx, idx + 1)
                            chunk_hook((gi0 + idx * JB) // P)
                        cur = nxt
                    if chunk_hook is not None:
                        norm_chunk(nch - 1, nch)
                        chunk_hook((gi0 + (nch - 1) * JB) // P)
                    else:
                        norm_chunk(0, nch)

                # ---- schedule ---------------------------------------------
                # mand supply is deferred as late as dependencies allow so
                # the big late i-blocks have filler; proj t12-15 are emitted
                # by the chunked-normalize hook inside att(1,3).
                POPN = [4, 2, 1, 1]

                def tail_hook(t_):
                    for g in unit_proj(t_, tail=True):
                        g()

                for ib in range(N_TB):
                    if ib == 0:
                        for wi in range(2):
                            mand.extend(unit_qk(1, wi, 0))
                        for t_ in range(2, 4):
                            mand.extend(unit_v(t_))
                        for hp in range(2):
                            for wi in range(2):
                                mand.extend(unit_qk(hp, wi, 1))
                        for t_ in range(4, 8):
                            mand.extend(unit_v(t_))
                    elif ib == 1:
                        for hp in range(2):
                            for wi in range(2):
                                mand.extend(unit_qk(hp, wi, 2))
                    elif ib == 2:
                        for t_ in range(8, 12):
                            mand.extend(unit_v(t_))
                        for hp in range(2):
                            mand.extend(unit_qk(hp, 1, 3))
                    else:
                        for hp in range(2):
                            mand.extend(unit_qk(hp, 0, 3))
                        for t_ in range(12, 16):
                            mand.extend(unit_v(t_))
                    if ib >= 1:
                        for t_ in range(4 * (ib - 1), 4 * ib):
                            flex.extend(unit_proj(t_))
                    reserve = 2 if ib == 2 else 0
                    aflex = ib >= 1
                    attention(0, ib, POPN[ib], allow_flex=aflex,
                              flex_reserve=reserve)
                    pop(2, allow_flex=aflex, flex_reserve=reserve)
                    attention(1, ib, POPN[ib], allow_flex=aflex,
                              flex_reserve=reserve,
                              chunk_hook=tail_hook if ib == 3 else None)
                    pop(2, allow_flex=aflex, flex_reserve=reserve)
                    flush_mand()
                while flex:
                    flex.popleft()()

    nc.compile()
    return nc


_NC = None


def _get_nc():
    global _NC
    if _NC is None:
        _NC = _build_nc()
    return _NC


def _warr(w):
    """W slice [HDL, E] -> SBUF layout [P, N_EC, HDL]: element (p, c, f) =
    W.T[c*P + p, f]."""
    return np.ascontiguousarray(
        w.T.reshape(N_EC, P, HDL).transpose(1, 0, 2)).astype(ml_dtypes.bfloat16)


def kernel(x, Wq, bq, Wk, bk, Wv, bv, Wp, bp, **_run_kwargs):
    x = np.asarray(x, dtype=np.float32)
    Wq = np.asarray(Wq, dtype=np.float32)
    Wk = np.asarray(Wk, dtype=np.float32)
    Wv = np.asarray(Wv, dtype=np.float32)
    Wp = np.asarray(Wp, dtype=np.float32)
    bq = np.asarray(bq, dtype=np.float32)
    bk = np.asarray(bk, dtype=np.float32)
    bv = np.asarray(bv, dtype=np.float32)
    bp = np.asarray(bp, dtype=np.float32)

    bp4 = (bp / GROUPS).astype(np.float32)

    in_maps = []
    for c in range(NCORES):
        b, hg = divmod(c, GROUPS)
        hsl = slice(HDL * hg, HDL * (hg + 1))
        in_maps.append({
            "xt4": np.ascontiguousarray(
                x[b].T.reshape(2, 4, P, N_TB, TQ).transpose(3, 0, 2, 1, 4)
            ).astype(ml_dtypes.bfloat16),
            "wqt": _warr(Wq[hsl]),
            "wkt": _warr(Wk[hsl]),
            "wvt": _warr(Wv[hsl]),
            "wpt": np.ascontiguousarray(
                Wp[:, hsl].T.reshape(2, P, E).transpose(1, 0, 2)
            ).astype(ml_dtypes.bfloat16),
            "bqv": np.ascontiguousarray(bq[hsl]),
            "bkv": np.ascontiguousarray(bk[hsl]),
            "bvv": np.ascontiguousarray(bv[hsl]),
            "bp4": bp4,
        })

    nc = _get_nc()
    try:
        res = run_bass_kernel_spmd(nc, in_maps, core_ids=list(range(NCORES)),
                                   **_run_kwargs)
    except Exception:
        # transient device hiccups (e.g. NRT_EXEC_UNIT_UNRECOVERABLE) have
        # been observed to clear on retry
        import time
        time.sleep(2.0)
        res = run_bass_kernel_spmd(nc, in_maps, core_ids=list(range(NCORES)),
                                   **_run_kwargs)
    outs = [r["out"].astype(np.float32) for r in res.results]
    y = np.stack([
        outs[0] + outs[1] + outs[2] + outs[3],
        outs[4] + outs[5] + outs[6] + outs[7],
    ]).astype(np.float32)
    if _run_kwargs:
        return y, res
    return y
